# revision 7
# baseline (speedup 1.0000x reference)
"""Trainium2 Bass kernel for nn_Message_gcn (2-layer RGCN + attention HypergraphConv + info-exchange MLP).

Sharding: pure data parallelism — batch 32 split as 4 samples on each of 8 NeuronCores,
per-layer weights replicated on every core.

Per-core algorithm (mathematically identical to the reference, restructured for the PE):
  - attention logits a_n / a_e computed via host-folded vectors u_x/u_e ([C,HH] = w_lin
    reshaped * att summed over the output channel), so the [M,C]@[C,4C] "el" matmul is
    never materialized.
  - softmax over incident hyperedges runs un-masked (max over all entries) and the mask
    is applied multiplicatively after exp() — identical by shift invariance since the
    global hyperedge keeps every row non-empty.
  - 1/deg, 1/|e|, 0.25/D(v) are folded into the adjacency columns / alpha operands so
    aggregations are plain matmuls with fp32 PSUM accumulation.
  - matmul operands use float32r (full-rate fp32 on the PE); the tiny info-exchange MLP
    (2 x [1024x1024] weights per layer, batched over the 4 samples) runs in bf16.
"""

import sys

sys.path.insert(0, "/opt/trn_rl_repo")

from contextlib import ExitStack

import numpy as np
import ml_dtypes

import concourse.bass as bass
import concourse.tile as tile
from concourse import bacc, mybir
from concourse.bass_utils import run_bass_kernel_spmd

BS, N, E, C, HH, L = 32, 256, 64, 512, 4, 2
M = E + 1
NCORES = 8
BSL = BS // NCORES          # samples per core
NB = N // 128               # node partition tiles
CT = C // 128               # channel partition tiles
C2 = 2 * C
KT2 = C2 // 128             # 2C partition tiles (ie)

f32 = mybir.dt.float32
f32r = mybir.dt.float32r
bf16 = mybir.dt.bfloat16
i32 = mybir.dt.int32
AF = mybir.ActivationFunctionType
ALU = mybir.AluOpType
AX = mybir.AxisListType


def _ins0(sl: bass.AP, count: int, pos: int) -> bass.AP:
    """Insert a 0-stride (broadcast) dim of `count` into an AP's free dims at
    position `pos` (0 = right after the partition dim, -1 = innermost)."""
    ap = [list(p) for p in sl.ap]
    if pos == -1:
        pos = len(ap) - 1
    ap.insert(1 + pos, [0, count])
    return bass.AP(tensor=sl.tensor, offset=sl.offset, ap=ap)


def _rep(sl: bass.AP, count: int) -> bass.AP:
    """[P, 1] AP -> [P, count] via 0-stride repeat of the free dim."""
    ap = [list(p) for p in sl.ap]
    assert ap[-1][1] == 1
    ap[-1] = [0, count]
    return bass.AP(tensor=sl.tensor, offset=sl.offset, ap=ap)


def _zeros_ap(sl: bass.AP, shape) -> bass.AP:
    """All-0-stride AP (scalar broadcast to `shape`, partition dim first)."""
    return bass.AP(tensor=sl.tensor, offset=sl.offset, ap=[[0, n] for n in shape])


def build_module():
    nc = bacc.Bacc("TRN2", target_bir_lowering=False, debug=False)

    # ---- DRAM I/O ----
    d_x0T = nc.dram_tensor("x0T", [BSL, C, N], f32r, kind="ExternalInput")
    d_eaT = nc.dram_tensor("eaT", [BSL, C, M], f32r, kind="ExternalInput")
    d_s2w = nc.dram_tensor("s2w", [BSL, N, E], i32, kind="ExternalInput")
    d_aug = nc.dram_tensor("aug", [BSL, N, N], i32, kind="ExternalInput")
    d_pun = nc.dram_tensor("pun", [BSL, N, N], i32, kind="ExternalInput")
    d_wlin = nc.dram_tensor("wlin", [L, C, HH * C], f32r, kind="ExternalInput")
    d_ux = nc.dram_tensor("ux", [L, C, HH], f32r, kind="ExternalInput")
    d_ue = nc.dram_tensor("ue", [L, C, HH], f32r, kind="ExternalInput")
    d_wcat = nc.dram_tensor("wcat", [L, C, 3 * C], f32r, kind="ExternalInput")
    d_iw1 = nc.dram_tensor("iw1", [L, C2, C2], bf16, kind="ExternalInput")
    d_iw2 = nc.dram_tensor("iw2", [L, C2, C2], bf16, kind="ExternalInput")
    d_brg = nc.dram_tensor("brg", [L, C], f32r, kind="ExternalInput")
    d_bhg = nc.dram_tensor("bhg", [L, C], f32r, kind="ExternalInput")
    d_ib1 = nc.dram_tensor("ib1", [L, C2], bf16, kind="ExternalInput")
    d_ib2 = nc.dram_tensor("ib2", [L, C2], bf16, kind="ExternalInput")
    # host constants (memset cannot write f32r)
    d_ones = nc.dram_tensor("onesc", [1, 128], f32r, kind="ExternalInput")
    d_eyer = nc.dram_tensor("eyer", [128, 128], f32r, kind="ExternalInput")
    d_eyeb = nc.dram_tensor("eyeb", [128, 128], bf16, kind="ExternalInput")
    d_onesb = nc.dram_tensor("onesb", [1, 4], bf16, kind="ExternalInput")
    d_outr = nc.dram_tensor("outr", [BSL, N, C], f32, kind="ExternalOutput")
    d_outh = nc.dram_tensor("outh", [BSL, N, C], f32, kind="ExternalOutput")

    with ExitStack() as ctx:
        tc = ctx.enter_context(tile.TileContext(nc))
        const = ctx.enter_context(tc.tile_pool(name="const", bufs=1))
        wts = ctx.enter_context(tc.tile_pool(name="wts", bufs=1))
        xst = ctx.enter_context(tc.tile_pool(name="xst", bufs=9))
        graph = ctx.enter_context(tc.tile_pool(name="graph", bufs=BSL))
        p1 = ctx.enter_context(tc.tile_pool(name="p1", bufs=1))
        p2 = ctx.enter_context(tc.tile_pool(name="p2", bufs=2))
        ps = ctx.enter_context(tc.tile_pool(name="ps", bufs=7, space="PSUM"))

        # ---- constants ----
        ones_row = const.tile([1, 128], f32r)
        nc.sync.dma_start(ones_row[:], d_ones[:])
        ones_col = const.tile([128, 1], f32r)
        nc.sync.dma_start(ones_col[:], d_ones[0:1, :].rearrange("o p -> p o"))
        identr = const.tile([128, 128], f32)
        nc.sync.dma_start(identr[:], d_eyer[:].bitcast(f32))
        identb = const.tile([128, 128], bf16)
        nc.sync.dma_start(identb[:], d_eyeb[:])
        ones4b = const.tile([1, 4], bf16)
        nc.sync.dma_start(ones4b[:], d_onesb[:])

        # ---- per-sample, layer-invariant graph state ----
        Hincs, invDqs, invBs, eaTs, x0Ts = [], [], [], [], []
        for s in range(BSL):
            x0T_t = xst.tile([128, CT, N], f32r, tag="xst")
            nc.sync.dma_start(x0T_t[:], d_x0T[s].rearrange("(ct p) n -> p ct n", p=128))
            x0Ts.append(x0T_t)

            eaT_t = graph.tile([128, CT, M], f32r)
            nc.sync.dma_start(eaT_t[:], d_eaT[s].rearrange("(ct p) m -> p ct m", p=128))
            eaTs.append(eaT_t)

            Hinc_t = graph.tile([128, NB, M], f32r)
            # col 0 (global hyperedge) = 1.0, cols 1..M-1 = sent2word incidence
            nc.sync.dma_start(Hinc_t[:, :, 0:1], _zeros_ap(d_ones[0:1, 0:1], [128, NB, 1]))
            nc.gpsimd.dma_start(Hinc_t[:, :, 1:M], d_s2w[s].rearrange("(t p) e -> p t e", p=128))
            Hincs.append(Hinc_t)

            # node degree -> 0.25/D(v) (head-mean folded in)
            Dn = p2.tile([128, NB], f32, tag="Dn")
            nc.vector.tensor_reduce(Dn[:], Hinc_t[:], axis=AX.X, op=ALU.add)
            eqD = p2.tile([128, NB], f32, tag="eqD")
            nc.vector.tensor_scalar(eqD[:], Dn[:], 0.0, None, op0=ALU.is_equal)
            invDq_t = graph.tile([128, NB], f32)
            nc.vector.tensor_add(invDq_t[:], Dn[:], eqD[:])
            nc.vector.reciprocal(invDq_t[:], invDq_t[:])
            nc.vector.tensor_sub(invDq_t[:], invDq_t[:], eqD[:])
            nc.vector.tensor_scalar(invDq_t[:], invDq_t[:], 0.25, None, op0=ALU.mult)
            invDqs.append(invDq_t)

            # hyperedge size -> 1/|e|
            Be_ps = ps.tile([M, 1], f32, tag="ps")
            for it in range(NB):
                nc.tensor.matmul(Be_ps[:], Hinc_t[:, it, :].bitcast(f32), ones_col[:].bitcast(f32),
                                 start=(it == 0), stop=(it == NB - 1))
            Be = p2.tile([M, 1], f32, tag="Be")
            nc.vector.tensor_copy(Be[:], Be_ps[:])
            eqB = p2.tile([M, 1], f32, tag="eqB")
            nc.vector.tensor_scalar(eqB[:], Be[:], 0.0, None, op0=ALU.is_equal)
            invB_t = graph.tile([M, 1], f32)
            nc.vector.tensor_add(invB_t[:], Be[:], eqB[:])
            nc.vector.reciprocal(invB_t[:], invB_t[:])
            nc.vector.tensor_sub(invB_t[:], invB_t[:], eqB[:])
            invBs.append(invB_t)

        xrTs = list(x0Ts)
        xhTs = list(x0Ts)

        for l in range(L):
            # ---- layer weights ----
            wlin_t = wts.tile([128, CT, HH * C], f32r, tag="wlin")
            nc.sync.dma_start(wlin_t[:], d_wlin[l].rearrange("(ct p) k -> p ct k", p=128))
            ux_t = wts.tile([128, CT, HH], f32r, tag="ux")
            nc.sync.dma_start(ux_t[:], d_ux[l].rearrange("(ct p) h -> p ct h", p=128))
            ue_t = wts.tile([128, CT, HH], f32r, tag="ue")
            nc.sync.dma_start(ue_t[:], d_ue[l].rearrange("(ct p) h -> p ct h", p=128))
            wcat_t = wts.tile([128, CT, 3 * C], f32r, tag="wcat")
            nc.sync.dma_start(wcat_t[:], d_wcat[l].rearrange("(ct p) k -> p ct k", p=128))
            iw1_t = wts.tile([128, KT2, C2], bf16, tag="iw1")
            nc.sync.dma_start(iw1_t[:], d_iw1[l].rearrange("(kt p) k -> p kt k", p=128))
            iw2_t = wts.tile([128, KT2, C2], bf16, tag="iw2")
            nc.sync.dma_start(iw2_t[:], d_iw2[l].rearrange("(kt p) k -> p kt k", p=128))
            brg_row = wts.tile([1, C], f32r, tag="brg")
            nc.sync.dma_start(brg_row[:], d_brg[l : l + 1, :])
            bhg_row = wts.tile([1, C], f32r, tag="bhg")
            nc.sync.dma_start(bhg_row[:], d_bhg[l : l + 1, :])
            ib1_row = wts.tile([1, C2], bf16, tag="ib1")
            nc.sync.dma_start(ib1_row[:], d_ib1[l : l + 1, :])
            ib2_row = wts.tile([1, C2], bf16, tag="ib2")
            nc.sync.dma_start(ib2_row[:], d_ib2[l : l + 1, :])

            # bias rows broadcast to [128, C] once per layer
            bb_ps = ps.tile([128, C], f32, tag="ps")
            nc.tensor.matmul(bb_ps[:], ones_row[:], brg_row[:], start=True, stop=True)
            brg_bc = wts.tile([128, C], f32, tag="brgbc")
            nc.vector.tensor_copy(brg_bc[:], bb_ps[:])
            bb2_ps = ps.tile([128, C], f32, tag="ps")
            nc.tensor.matmul(bb2_ps[:], ones_row[:], bhg_row[:], start=True, stop=True)
            bhg_bc = wts.tile([128, C], f32, tag="bhgbc")
            nc.scalar.copy(bhg_bc[:], bb2_ps[:])

            ctxT = p1.tile([128, 2 * CT, BSL], bf16, tag="ctxT")
            for s in range(BSL):
                xrT = xrTs[s]
                xhT = xhTs[s]

                # ---- typed adjacency A' with 1/deg folded into its columns ----
                Af = p1.tile([128, 2, NB, N], f32r, tag="Af")
                nc.gpsimd.dma_start(Af[:, 1, :, :], d_aug[s].rearrange("(t p) j -> p t j", p=128))
                nc.gpsimd.dma_start(Af[:, 0, :, :], d_pun[s].rearrange("(t p) j -> p t j", p=128))
                onem = p1.tile([128, NB, N], f32, tag="gtmp")
                nc.vector.tensor_scalar(onem[:], Af[:, 1, :, :], -1.0, 1.0, op0=ALU.mult, op1=ALU.add)
                nc.vector.tensor_mul(Af[:, 0, :, :], Af[:, 0, :, :], onem[:])

                deg_ps = ps.tile([1, 2, N], f32, tag="ps")
                for r in range(2):
                    for it in range(NB):
                        nc.tensor.matmul(deg_ps[:, r, :], ones_col[:], Af[:, r, it, :],
                                         start=(it == 0), stop=(it == NB - 1))
                deg_sb = p1.tile([1, 2, N], f32r, tag="degsb")
                nc.vector.tensor_copy(deg_sb[:], deg_ps[:])
                dbc_ps = ps.tile([128, 2, N], f32, tag="ps")
                for r in range(2):
                    nc.tensor.matmul(dbc_ps[:, r, :], ones_row[:], deg_sb[:, r, :],
                                     start=True, stop=True)
                eqd = p1.tile([128, 2, N], f32, tag="eqd")
                nc.vector.tensor_scalar(eqd[:], dbc_ps[:], 0.0, None, op0=ALU.is_equal)
                invd = p1.tile([128, 2, N], f32, tag="invd")
                nc.vector.tensor_add(invd[:], dbc_ps[:], eqd[:])
                nc.vector.reciprocal(invd[:], invd[:])
                nc.vector.tensor_sub(invd[:], invd[:], eqd[:])
                for r in range(2):
                    for it in range(NB):
                        nc.vector.tensor_mul(Af[:, r, it, :], Af[:, r, it, :], invd[:, r, :])

                # ---- attention logits ----
                an_ps = ps.tile([128, NB * HH], f32, tag="ps")
                for nb in range(NB):
                    for ct in range(CT):
                        nc.tensor.matmul(an_ps[:, nb * HH : (nb + 1) * HH],
                                         xhT[:, ct, nb * 128 : (nb + 1) * 128].bitcast(f32),
                                         ux_t[:, ct, :].bitcast(f32),
                                         start=(ct == 0), stop=(ct == CT - 1))
                an_sb = p2.tile([128, NB * HH], f32, tag="ansb")
                nc.vector.tensor_copy(an_sb[:], an_ps[:])

                # one [1, M] row per head, all based at partition 0 (matmul base-partition rule)
                ae_ps = ps.tile([1, HH, M], f32, tag="ps")
                for h in range(HH):
                    for ct in range(CT):
                        nc.tensor.matmul(ae_ps[:, h, :], ue_t[:, ct, h : h + 1].bitcast(f32),
                                         eaTs[s][:, ct, :].bitcast(f32),
                                         start=(ct == 0), stop=(ct == CT - 1))
                ae_sb = p2.tile([1, HH, M], f32, tag="aesb")
                nc.vector.tensor_copy(ae_sb[:], ae_ps[:])

                # a_e broadcast along nodes (shared by both node blocks)
                ab_ps = ps.tile([128, HH, M], f32, tag="ps")
                for h in range(HH):
                    nc.tensor.matmul(ab_ps[:, h, :], ones_row[:].bitcast(f32), ae_sb[:, h, :],
                                     start=True, stop=True)

                # ---- alpha (leaky relu + masked softmax), invD/invB folded variants ----
                alpha = p1.tile([128, NB, HH, M], f32r, tag="alpha")
                alpha3T = p1.tile([M, HH, N], f32r, tag="alpha3T")
                for nb in range(NB):
                    t1 = p1.tile([128, HH, M], f32, tag="t1")
                    nc.vector.tensor_tensor(t1[:], ab_ps[:],
                                            _ins0(an_sb[:, nb * HH : (nb + 1) * HH], M, -1),
                                            op=ALU.add)
                    nc.vector.scalar_tensor_tensor(t1[:], t1[:], 0.2, t1[:],
                                                   op0=ALU.mult, op1=ALU.max)
                    nmax = p2.tile([128, HH], f32, tag="nmax")
                    nc.vector.tensor_reduce(nmax[:], t1[:], axis=AX.X, op=ALU.max, negate=True)
                    nc.vector.tensor_tensor(t1[:], t1[:], _ins0(nmax[:], M, -1), op=ALU.add)
                    nc.scalar.activation(t1[:], t1[:], AF.Exp)
                    nc.vector.tensor_tensor(t1[:], t1[:], _ins0(Hincs[s][:, nb, :], HH, 0),
                                            op=ALU.mult)
                    ssum = p2.tile([128, HH], f32, tag="ssum")
                    nc.vector.tensor_reduce(ssum[:], t1[:], axis=AX.X, op=ALU.add)
                    rs = p2.tile([128, HH], f32, tag="rs")
                    nc.vector.reciprocal(rs[:], ssum[:])
                    rcol2 = p2.tile([128, HH], f32, tag="rcol2")
                    nc.vector.tensor_tensor(rcol2[:], rs[:], _rep(invDqs[s][:, nb : nb + 1], HH),
                                            op=ALU.mult)
                    nc.vector.tensor_tensor(alpha[:, nb, :, :], t1[:], _ins0(rs[:], M, -1),
                                            op=ALU.mult)
                    nc.vector.tensor_tensor(t1[:], t1[:], _ins0(rcol2[:], M, -1), op=ALU.mult)
                    for h in range(HH):
                        aT_ps = ps.tile([M, 128], f32, tag="ps")
                        nc.tensor.transpose(aT_ps[:], t1[:, h, :], identr[:])
                        nc.vector.tensor_scalar(alpha3T[:, h, nb * 128 : (nb + 1) * 128],
                                                aT_ps[:], invBs[s][:, 0:1], None, op0=ALU.mult)

                # ---- hypergraph conv: xl per head -> msg -> out_h ----
                msg = p1.tile([M, HH, C], f32r, tag="msg")
                for h in range(HH):
                    xlh = p2.tile([128, NB, C], f32r, tag="xlh")
                    for nb in range(NB):
                        xl_ps = ps.tile([128, C], f32, tag="ps")
                        for ct in range(CT):
                            nc.tensor.matmul(xl_ps[:],
                                             xhT[:, ct, nb * 128 : (nb + 1) * 128],
                                             wlin_t[:, ct, h * C : (h + 1) * C],
                                             start=(ct == 0), stop=(ct == CT - 1))
                        if h % 2 == 0:
                            nc.vector.tensor_copy(xlh[:, nb, :], xl_ps[:])
                        else:
                            nc.scalar.copy(xlh[:, nb, :], xl_ps[:])
                    msg_ps = ps.tile([M, C], f32, tag="ps")
                    for nb in range(NB):
                        nc.tensor.matmul(msg_ps[:], alpha[:, nb, h, :], xlh[:, nb, :],
                                         start=(nb == 0), stop=(nb == NB - 1))
                    if h % 2 == 0:
                        nc.scalar.copy(msg[:, h, :], msg_ps[:])
                    else:
                        nc.vector.tensor_copy(msg[:, h, :], msg_ps[:])

                outh_t = p1.tile([128, NB, C], f32r, tag="outh_t")
                for nb in range(NB):
                    oh_ps = ps.tile([128, C], f32, tag="ps")
                    for h in range(HH):
                        nc.tensor.matmul(oh_ps[:], alpha3T[:, h, nb * 128 : (nb + 1) * 128],
                                         msg[:, h, :], start=(h == 0), stop=(h == HH - 1))
                    nc.vector.tensor_add(oh_ps[:], oh_ps[:], bhg_bc[:])
                    nc.scalar.activation(outh_t[:, nb, :], oh_ps[:], AF.Relu)

                # ---- RGCN: xw per relation -> aggregate + root + bias ----
                xw = p1.tile([128, NB, 2, C], f32r, tag="xw")
                for r in range(2):
                    for nb in range(NB):
                        xw_ps = ps.tile([128, C], f32, tag="ps")
                        for ct in range(CT):
                            nc.tensor.matmul(xw_ps[:],
                                             xrT[:, ct, nb * 128 : (nb + 1) * 128],
                                             wcat_t[:, ct, r * C : (r + 1) * C],
                                             start=(ct == 0), stop=(ct == CT - 1))
                        if (r + nb) % 2 == 0:
                            nc.vector.tensor_copy(xw[:, nb, r, :], xw_ps[:])
                        else:
                            nc.scalar.copy(xw[:, nb, r, :], xw_ps[:])

                outr_t = p1.tile([128, NB, C], f32r, tag="outr_t")
                for jb in range(NB):
                    or_ps = ps.tile([128, C], f32, tag="ps")
                    for ct in range(CT):
                        nc.tensor.matmul(or_ps[:],
                                         xrT[:, ct, jb * 128 : (jb + 1) * 128],
                                         wcat_t[:, ct, 2 * C : 3 * C],
                                         start=(ct == 0), stop=False)
                    for r in range(2):
                        for it in range(NB):
                            nc.tensor.matmul(or_ps[:],
                                             Af[:, r, it, jb * 128 : (jb + 1) * 128],
                                             xw[:, it, r, :],
                                             start=False, stop=((r, it) == (1, NB - 1)))
                    nc.vector.tensor_add(or_ps[:], or_ps[:], brg_bc[:])
                    nc.scalar.activation(outr_t[:, jb, :], or_ps[:], AF.Relu)

                # ---- gather ctx rows (row 0 of out_r / out_h) into bf16 ctxT columns ----
                ctx_psr = ps.tile([128, CT], f32, tag="ps")
                ctx_psh = ps.tile([128, CT], f32, tag="ps")
                for ct in range(CT):
                    nc.tensor.transpose(ctx_psr[:, ct : ct + 1],
                                        outr_t[0:1, 0, ct * 128 : (ct + 1) * 128].bitcast(f32),
                                        identr[0:1, 0:1])
                    nc.tensor.transpose(ctx_psh[:, ct : ct + 1],
                                        outh_t[0:1, 0, ct * 128 : (ct + 1) * 128].bitcast(f32),
                                        identr[0:1, 0:1])
                nc.vector.tensor_copy(ctxT[:, 0:CT, s], ctx_psr[:])
                nc.vector.tensor_copy(ctxT[:, CT : 2 * CT, s], ctx_psh[:])

                # ---- transpose outputs into next-layer feature-major state (layer 0) ----
                if l == 0:
                    xrT1 = xst.tile([128, CT, N], f32r, tag="xst")
                    xhT1 = xst.tile([128, CT, N], f32r, tag="xst")
                    for src, dst in ((outr_t, xrT1), (outh_t, xhT1)):
                        for nb in range(NB):
                            for ct in range(CT):
                                tp_ps = ps.tile([128, 128], f32, tag="ps")
                                nc.tensor.transpose(tp_ps[:],
                                                    src[:, nb, ct * 128 : (ct + 1) * 128].bitcast(f32),
                                                    identr[:])
                                if (nb + ct) % 2 == 0:
                                    nc.vector.tensor_copy(dst[:, ct, nb * 128 : (nb + 1) * 128], tp_ps[:])
                                else:
                                    nc.scalar.copy(dst[:, ct, nb * 128 : (nb + 1) * 128], tp_ps[:])
                    xrTs[s] = xrT1
                    xhTs[s] = xhT1
                else:
                    # final layer: store rows 1..N-1 (row 0 comes from the ie MLP below)
                    for tsrc, dram in ((outr_t, d_outr), (outh_t, d_outh)):
                        nc.sync.dma_start(dram[s, 1:128, :].bitcast(f32r), tsrc[1:128, 0, :])
                        nc.sync.dma_start(dram[s, 128:N, :].bitcast(f32r), tsrc[:, 1, :])

            # ---- info-exchange MLP over the 4 samples' ctx rows (bf16) ----
            y1 = p1.tile([BSL, C2], bf16, tag="y1")
            for ch in range(2):
                ie_ps = ps.tile([BSL, C], f32, tag="ps")
                for kt in range(KT2):
                    nc.tensor.matmul(ie_ps[:], ctxT[:, kt, :], iw1_t[:, kt, ch * C : (ch + 1) * C],
                                     start=(kt == 0), stop=False)
                nc.tensor.matmul(ie_ps[:], ones4b[:], ib1_row[:, ch * C : (ch + 1) * C],
                                 start=False, stop=True)
                nc.scalar.activation(y1[:, ch * C : (ch + 1) * C], ie_ps[:], AF.Relu)
            ctx2T = p1.tile([128, KT2, BSL], bf16, tag="ctx2T")
            for kt in range(KT2):
                t2_ps = ps.tile([128, BSL], bf16, tag="ps")
                nc.tensor.transpose(t2_ps[:], y1[:, kt * 128 : (kt + 1) * 128],
                                    identb[0:BSL, 0:BSL])
                nc.vector.tensor_copy(ctx2T[:, kt, :], t2_ps[:])
            y2 = p1.tile([BSL, C2], f32, tag="y2")
            for ch in range(2):
                ie2_ps = ps.tile([BSL, C], f32, tag="ps")
                for kt in range(KT2):
                    nc.tensor.matmul(ie2_ps[:], ctx2T[:, kt, :], iw2_t[:, kt, ch * C : (ch + 1) * C],
                                     start=(kt == 0), stop=False)
                nc.tensor.matmul(ie2_ps[:], ones4b[:], ib2_row[:, ch * C : (ch + 1) * C],
                                 start=False, stop=True)
                nc.vector.tensor_copy(y2[:, ch * C : (ch + 1) * C], ie2_ps[:])

            if l == 0:
                # scatter y2 rows back as column 0 of the next-layer feature-major states
                for kt in range(KT2):
                    y2T_ps = ps.tile([128, BSL], f32, tag="ps")
                    nc.tensor.transpose(y2T_ps[:], y2[:, kt * 128 : (kt + 1) * 128],
                                        identr[0:BSL, 0:BSL])
                    for s in range(BSL):
                        dst = xrTs[s] if kt < CT else xhTs[s]
                        nc.vector.tensor_copy(dst[:, kt % CT, 0:1], y2T_ps[:, s : s + 1])
            else:
                for s in range(BSL):
                    nc.sync.dma_start(d_outr[s, 0:1, :], y2[s : s + 1, 0:C])
                    nc.sync.dma_start(d_outh[s, 0:1, :], y2[s : s + 1, C:C2])

    nc.compile()
    return nc


_NC = None


def _get_nc():
    global _NC
    if _NC is None:
        _NC = build_module()
    return _NC


def make_in_maps(encoded_spans, SVO_emb, pooled_output, sent2word_adj, aug_adj,
                 punct_graph, w_rel, w_root, b_rgcn, w_lin, att_x, att_e, b_hgcn,
                 ie_w1, ie_b1, ie_w2, ie_b2):
    f = np.float32
    bf = ml_dtypes.bfloat16
    # host-folded attention vectors: u[c,h] = sum_k w_lin[c, h*C+k] * att[h,k]
    wl = np.ascontiguousarray(np.asarray(w_lin, f))                # [L, C, HH*C]
    wl4 = wl.reshape(L, C, HH, C)
    ux = np.einsum("lchk,lhk->lch", wl4, np.asarray(att_x, f))     # [L, C, HH]
    ue = np.einsum("lchk,lhk->lch", wl4, np.asarray(att_e, f))
    wr = np.asarray(w_rel, f)
    wcat = np.concatenate([wr[:, 0], wr[:, 1], np.asarray(w_root, f)], axis=2)
    e_attr = np.concatenate([np.asarray(pooled_output, f)[:, None, :],
                             np.asarray(SVO_emb, f)], axis=1)      # [BS, M, C]
    eaT = np.ascontiguousarray(e_attr.transpose(0, 2, 1))          # [BS, C, M]
    x0T = np.ascontiguousarray(np.asarray(encoded_spans, f).transpose(0, 2, 1))

    shared = {
        "wlin": wl,
        "ux": np.ascontiguousarray(ux),
        "ue": np.ascontiguousarray(ue),
        "wcat": np.ascontiguousarray(wcat),
        "iw1": np.asarray(ie_w1, f).astype(bf),
        "iw2": np.asarray(ie_w2, f).astype(bf),
        "brg": np.asarray(b_rgcn, f),
        "bhg": np.asarray(b_hgcn, f),
        "ib1": np.asarray(ie_b1, f).astype(bf),
        "ib2": np.asarray(ie_b2, f).astype(bf),
        "onesc": np.ones((1, 128), f),
        "eyer": np.eye(128, dtype=f),
        "eyeb": np.eye(128, dtype=f).astype(bf),
        "onesb": np.ones((1, 4), f).astype(bf),
    }
    s2w = np.ascontiguousarray(np.asarray(sent2word_adj, np.int32))
    aug = np.ascontiguousarray(np.asarray(aug_adj, np.int32))
    pun = np.ascontiguousarray(np.asarray(punct_graph, np.int32))

    in_maps = []
    for c in range(NCORES):
        sl = slice(c * BSL, (c + 1) * BSL)
        m = dict(shared)
        m["x0T"] = np.ascontiguousarray(x0T[sl])
        m["eaT"] = np.ascontiguousarray(eaT[sl])
        m["s2w"] = s2w[sl]
        m["aug"] = aug[sl]
        m["pun"] = pun[sl]
        in_maps.append(m)
    return in_maps


def run(in_maps, trace=False, **kw):
    nc = _get_nc()
    return run_bass_kernel_spmd(nc, in_maps, list(range(NCORES)), trace=trace, **kw)


def kernel(**inputs):
    in_maps = make_in_maps(**inputs)
    res = run(in_maps)
    x_r = np.concatenate([res.results[c]["outr"] for c in range(NCORES)], axis=0)
    x_h = np.concatenate([res.results[c]["outh"] for c in range(NCORES)], axis=0)
    return x_r.astype(np.float32), x_h.astype(np.float32)


# revision 9
# speedup vs baseline: 1.0517x; 1.0517x over previous
"""Trainium2 Bass kernel for nn_Message_gcn (2-layer RGCN + attention HypergraphConv + info-exchange MLP).

Sharding: pure data parallelism — batch 32 split as 4 samples on each of 8 NeuronCores,
per-layer weights replicated on every core.

Per-core algorithm (mathematically identical to the reference, restructured for the PE):
  - attention logits a_n / a_e computed via host-folded vectors u_x/u_e ([C,HH] = w_lin
    reshaped * att summed over the output channel), so the [M,C]@[C,4C] "el" matmul is
    never materialized.
  - softmax over incident hyperedges runs un-masked (max over all entries) and the mask
    is applied multiplicatively after exp() — identical by shift invariance since the
    global hyperedge keeps every row non-empty.
  - 1/deg, 1/|e|, 0.25/D(v) are folded into the adjacency columns / alpha operands so
    aggregations are plain matmuls with fp32 PSUM accumulation.
  - matmul operands use float32r (full-rate fp32 on the PE); the tiny info-exchange MLP
    (2 x [1024x1024] weights per layer, batched over the 4 samples) runs in bf16.
"""

import sys

sys.path.insert(0, "/opt/trn_rl_repo")

from contextlib import ExitStack

import numpy as np
import ml_dtypes

import concourse.bass as bass
import concourse.tile as tile
from concourse import bacc, mybir
from concourse.bass_utils import run_bass_kernel_spmd

BS, N, E, C, HH, L = 32, 256, 64, 512, 4, 2
M = E + 1
NCORES = 8
BSL = BS // NCORES          # samples per core
NB = N // 128               # node partition tiles
CT = C // 128               # channel partition tiles
C2 = 2 * C
KT2 = C2 // 128             # 2C partition tiles (ie)

f32 = mybir.dt.float32
f32r = mybir.dt.float32r
bf16 = mybir.dt.bfloat16
i32 = mybir.dt.int32
AF = mybir.ActivationFunctionType
ALU = mybir.AluOpType
AX = mybir.AxisListType


def _ins0(sl: bass.AP, count: int, pos: int) -> bass.AP:
    """Insert a 0-stride (broadcast) dim of `count` into an AP's free dims at
    position `pos` (0 = right after the partition dim, -1 = innermost)."""
    ap = [list(p) for p in sl.ap]
    if pos == -1:
        pos = len(ap) - 1
    ap.insert(1 + pos, [0, count])
    return bass.AP(tensor=sl.tensor, offset=sl.offset, ap=ap)


def _rep(sl: bass.AP, count: int) -> bass.AP:
    """[P, 1] AP -> [P, count] via 0-stride repeat of the free dim."""
    ap = [list(p) for p in sl.ap]
    assert ap[-1][1] == 1
    ap[-1] = [0, count]
    return bass.AP(tensor=sl.tensor, offset=sl.offset, ap=ap)


def _zeros_ap(sl: bass.AP, shape) -> bass.AP:
    """All-0-stride AP (scalar broadcast to `shape`, partition dim first)."""
    return bass.AP(tensor=sl.tensor, offset=sl.offset, ap=[[0, n] for n in shape])


def build_module():
    nc = bacc.Bacc("TRN2", target_bir_lowering=False, debug=False)

    # ---- DRAM I/O ----
    d_x0T = nc.dram_tensor("x0T", [BSL, C, N], f32r, kind="ExternalInput")
    d_eaT = nc.dram_tensor("eaT", [BSL, C, M], f32r, kind="ExternalInput")
    d_s2w = nc.dram_tensor("s2w", [BSL, N, E], i32, kind="ExternalInput")
    d_aug = nc.dram_tensor("aug", [BSL, N, N], i32, kind="ExternalInput")
    d_pun = nc.dram_tensor("pun", [BSL, N, N], i32, kind="ExternalInput")
    d_wlin = nc.dram_tensor("wlin", [L, C, HH * C], f32r, kind="ExternalInput")
    d_ux = nc.dram_tensor("ux", [L, C, HH], f32r, kind="ExternalInput")
    d_ue = nc.dram_tensor("ue", [L, C, HH], f32r, kind="ExternalInput")
    d_wcat = nc.dram_tensor("wcat", [L, C, 3 * C], f32r, kind="ExternalInput")
    d_iw1 = nc.dram_tensor("iw1", [L, C2, C2], bf16, kind="ExternalInput")
    d_iw2 = nc.dram_tensor("iw2", [L, C2, C2], bf16, kind="ExternalInput")
    d_brg = nc.dram_tensor("brg", [L, C], f32r, kind="ExternalInput")
    d_bhg = nc.dram_tensor("bhg", [L, C], f32r, kind="ExternalInput")
    d_ib1 = nc.dram_tensor("ib1", [L, C2], bf16, kind="ExternalInput")
    d_ib2 = nc.dram_tensor("ib2", [L, C2], bf16, kind="ExternalInput")
    # host constants (memset cannot write f32r)
    d_ones = nc.dram_tensor("onesc", [1, 128], f32r, kind="ExternalInput")
    d_eyer = nc.dram_tensor("eyer", [128, 128], f32r, kind="ExternalInput")
    d_eyeb = nc.dram_tensor("eyeb", [128, 128], bf16, kind="ExternalInput")
    d_onesb = nc.dram_tensor("onesb", [1, 4], bf16, kind="ExternalInput")
    d_outr = nc.dram_tensor("outr", [BSL, N, C], f32, kind="ExternalOutput")
    d_outh = nc.dram_tensor("outh", [BSL, N, C], f32, kind="ExternalOutput")

    with ExitStack() as ctx:
        tc = ctx.enter_context(tile.TileContext(nc))
        const = ctx.enter_context(tc.tile_pool(name="const", bufs=1))
        wts = ctx.enter_context(tc.tile_pool(name="wts", bufs=1))
        xst = ctx.enter_context(tc.tile_pool(name="xst", bufs=9))
        graph = ctx.enter_context(tc.tile_pool(name="graph", bufs=BSL))
        p1 = ctx.enter_context(tc.tile_pool(name="p1", bufs=1))
        p2 = ctx.enter_context(tc.tile_pool(name="p2", bufs=2))
        ps = ctx.enter_context(tc.tile_pool(name="ps", bufs=8, space="PSUM"))

        # ---- constants ----
        ones_row = const.tile([1, 128], f32r)
        nc.sync.dma_start(ones_row[:], d_ones[:])
        ones_col = const.tile([128, 1], f32r)
        nc.sync.dma_start(ones_col[:], d_ones[0:1, :].rearrange("o p -> p o"))
        identr = const.tile([128, 128], f32)
        nc.sync.dma_start(identr[:], d_eyer[:].bitcast(f32))
        identb = const.tile([128, 128], bf16)
        nc.sync.dma_start(identb[:], d_eyeb[:])
        ones4b = const.tile([1, 4], bf16)
        nc.sync.dma_start(ones4b[:], d_onesb[:])

        # ---- per-sample, layer-invariant graph state ----
        Hincs, invDqs, invBs, eaTs, x0Ts = [], [], [], [], []
        for s in range(BSL):
            x0T_t = xst.tile([128, CT, N], f32r, tag="xst")
            nc.scalar.dma_start(x0T_t[:], d_x0T[s].rearrange("(ct p) n -> p ct n", p=128))
            x0Ts.append(x0T_t)

            eaT_t = graph.tile([128, CT, M], f32r)
            nc.sync.dma_start(eaT_t[:], d_eaT[s].rearrange("(ct p) m -> p ct m", p=128))
            eaTs.append(eaT_t)

            Hinc_t = graph.tile([128, NB, M], f32r)
            # col 0 (global hyperedge) = 1.0, cols 1..M-1 = sent2word incidence
            nc.sync.dma_start(Hinc_t[:, :, 0:1], _zeros_ap(d_ones[0:1, 0:1], [128, NB, 1]))
            nc.gpsimd.dma_start(Hinc_t[:, :, 1:M], d_s2w[s].rearrange("(t p) e -> p t e", p=128))
            Hincs.append(Hinc_t)

            # node degree -> 0.25/D(v) (head-mean folded in)
            Dn = p2.tile([128, NB], f32, tag="Dn")
            nc.vector.tensor_reduce(Dn[:], Hinc_t[:], axis=AX.X, op=ALU.add)
            eqD = p2.tile([128, NB], f32, tag="eqD")
            nc.vector.tensor_scalar(eqD[:], Dn[:], 0.0, None, op0=ALU.is_equal)
            invDq_t = graph.tile([128, NB], f32)
            nc.vector.tensor_add(invDq_t[:], Dn[:], eqD[:])
            nc.vector.reciprocal(invDq_t[:], invDq_t[:])
            nc.vector.tensor_sub(invDq_t[:], invDq_t[:], eqD[:])
            nc.vector.tensor_scalar(invDq_t[:], invDq_t[:], 0.25, None, op0=ALU.mult)
            invDqs.append(invDq_t)

            # hyperedge size -> 1/|e|
            Be_ps = ps.tile([M, 1], f32, tag="ps")
            for it in range(NB):
                nc.tensor.matmul(Be_ps[:], Hinc_t[:, it, :].bitcast(f32), ones_col[:].bitcast(f32),
                                 start=(it == 0), stop=(it == NB - 1))
            Be = p2.tile([M, 1], f32, tag="Be")
            nc.vector.tensor_copy(Be[:], Be_ps[:])
            eqB = p2.tile([M, 1], f32, tag="eqB")
            nc.vector.tensor_scalar(eqB[:], Be[:], 0.0, None, op0=ALU.is_equal)
            invB_t = graph.tile([M, 1], f32)
            nc.vector.tensor_add(invB_t[:], Be[:], eqB[:])
            nc.vector.reciprocal(invB_t[:], invB_t[:])
            nc.vector.tensor_sub(invB_t[:], invB_t[:], eqB[:])
            invBs.append(invB_t)

        xrTs = list(x0Ts)
        xhTs = list(x0Ts)

        for l in range(L):
            # ---- layer weights ----
            wlin_t = wts.tile([128, CT, HH * C], f32r, tag="wlin")
            nc.scalar.dma_start(wlin_t[:], d_wlin[l].rearrange("(ct p) k -> p ct k", p=128))
            ux_t = wts.tile([128, CT, HH], f32r, tag="ux")
            nc.sync.dma_start(ux_t[:], d_ux[l].rearrange("(ct p) h -> p ct h", p=128))
            ue_t = wts.tile([128, CT, HH], f32r, tag="ue")
            nc.sync.dma_start(ue_t[:], d_ue[l].rearrange("(ct p) h -> p ct h", p=128))
            wcat_t = wts.tile([128, CT, 3 * C], f32r, tag="wcat")
            nc.sync.dma_start(wcat_t[:], d_wcat[l].rearrange("(ct p) k -> p ct k", p=128))
            iw1_t = wts.tile([128, KT2, C2], bf16, tag="iw1")
            nc.scalar.dma_start(iw1_t[:], d_iw1[l].rearrange("(kt p) k -> p kt k", p=128))
            iw2_t = wts.tile([128, KT2, C2], bf16, tag="iw2")
            nc.scalar.dma_start(iw2_t[:], d_iw2[l].rearrange("(kt p) k -> p kt k", p=128))
            brg_row = wts.tile([1, C], f32r, tag="brg")
            nc.sync.dma_start(brg_row[:], d_brg[l : l + 1, :])
            bhg_row = wts.tile([1, C], f32r, tag="bhg")
            nc.sync.dma_start(bhg_row[:], d_bhg[l : l + 1, :])
            ib1_row = wts.tile([1, C2], bf16, tag="ib1")
            nc.sync.dma_start(ib1_row[:], d_ib1[l : l + 1, :])
            ib2_row = wts.tile([1, C2], bf16, tag="ib2")
            nc.sync.dma_start(ib2_row[:], d_ib2[l : l + 1, :])

            ctxT = p1.tile([128, 2 * CT, BSL], bf16, tag="ctxT")
            for s in range(BSL):
                xrT = xrTs[s]
                xhT = xhTs[s]

                # ---- typed adjacency A' with 1/deg folded into its columns ----
                Af = p1.tile([128, 2, NB, N], f32r, tag="Af")
                nc.gpsimd.dma_start(Af[:, 1, :, :], d_aug[s].rearrange("(t p) j -> p t j", p=128))
                nc.gpsimd.dma_start(Af[:, 0, :, :], d_pun[s].rearrange("(t p) j -> p t j", p=128))
                onem = p1.tile([128, NB, N], f32, tag="gtmp")
                nc.vector.tensor_scalar(onem[:], Af[:, 1, :, :], -1.0, 1.0, op0=ALU.mult, op1=ALU.add)
                nc.vector.tensor_mul(Af[:, 0, :, :], Af[:, 0, :, :], onem[:])

                # in-degree rows -> transpose to per-target columns; guarded 1/deg
                deg_ps = ps.tile([1, 2, N], f32, tag="ps")
                for r in range(2):
                    for it in range(NB):
                        nc.tensor.matmul(deg_ps[:, r, :], ones_col[:], Af[:, r, it, :],
                                         start=(it == 0), stop=(it == NB - 1))
                degrow = p1.tile([1, 2, N], f32, tag="degrow")
                nc.scalar.copy(degrow[:], deg_ps[:])
                degc_ps = ps.tile([128, 2 * NB], f32, tag="ps")
                for r in range(2):
                    for jb in range(NB):
                        nc.tensor.transpose(degc_ps[:, r * NB + jb : r * NB + jb + 1],
                                            degrow[0:1, r, jb * 128 : (jb + 1) * 128],
                                            identr[0:1, 0:1])
                eqc = p2.tile([128, 2 * NB], f32, tag="eqc")
                nc.vector.tensor_scalar(eqc[:], degc_ps[:], 0.0, None, op0=ALU.is_equal)
                ivc = p2.tile([128, 2 * NB], f32, tag="ivc")
                nc.vector.tensor_add(ivc[:], degc_ps[:], eqc[:])
                nc.vector.reciprocal(ivc[:], ivc[:])
                nc.vector.tensor_sub(ivc[:], ivc[:], eqc[:])

                # ---- attention logits ----
                an_ps = ps.tile([128, NB * HH], f32, tag="ps")
                for nb in range(NB):
                    for ct in range(CT):
                        nc.tensor.matmul(an_ps[:, nb * HH : (nb + 1) * HH],
                                         xhT[:, ct, nb * 128 : (nb + 1) * 128].bitcast(f32),
                                         ux_t[:, ct, :].bitcast(f32),
                                         start=(ct == 0), stop=(ct == CT - 1))
                an_sb = p2.tile([128, NB * HH], f32, tag="ansb")
                nc.vector.tensor_copy(an_sb[:], an_ps[:])

                ae_ps = ps.tile([HH, M], f32, tag="ps")
                for ct in range(CT):
                    nc.tensor.matmul(ae_ps[:], ue_t[:, ct, :].bitcast(f32),
                                     eaTs[s][:, ct, :].bitcast(f32),
                                     start=(ct == 0), stop=(ct == CT - 1))
                ae4_sb = p2.tile([HH, M], f32r, tag="ae4sb")
                nc.vector.tensor_copy(ae4_sb[:], ae_ps[:])
                # gather the 4 head rows onto partition 0 (matmul base-partition rule)
                ae_row = p2.tile([1, HH, M], f32r, tag="aerow")
                for h in range(HH):
                    nc.scalar.dma_start(ae_row[:, h, :], ae4_sb[h : h + 1, :])
                # a_e broadcast along nodes in one K=1 fp32r matmul (shared by both node blocks)
                ab_ps = ps.tile([128, HH, M], f32, tag="ps")
                nc.tensor.matmul(ab_ps[:], ones_row[:], ae_row[0:1, :, :], start=True, stop=True)

                # ---- alpha (leaky relu + masked softmax), invD/invB folded variants ----
                alpha = p1.tile([128, NB, HH, M], f32r, tag="alpha")
                alpha3T = p1.tile([M, HH, N], f32r, tag="alpha3T")
                for nb in range(NB):
                    t1 = p1.tile([128, HH, M], f32, tag="t1")
                    nc.vector.tensor_tensor(t1[:], ab_ps[:],
                                            _ins0(an_sb[:, nb * HH : (nb + 1) * HH], M, -1),
                                            op=ALU.add)
                    nc.vector.scalar_tensor_tensor(t1[:], t1[:], 0.2, t1[:],
                                                   op0=ALU.mult, op1=ALU.max)
                    nmax = p2.tile([128, HH], f32, tag="nmax")
                    nc.vector.tensor_reduce(nmax[:], t1[:], axis=AX.X, op=ALU.max, negate=True)
                    nc.vector.tensor_tensor(t1[:], t1[:], _ins0(nmax[:], M, -1), op=ALU.add)
                    nc.scalar.activation(t1[:], t1[:], AF.Exp)
                    nc.vector.tensor_tensor(t1[:], t1[:], _ins0(Hincs[s][:, nb, :], HH, 0),
                                            op=ALU.mult)
                    ssum = p2.tile([128, HH], f32, tag="ssum")
                    nc.vector.tensor_reduce(ssum[:], t1[:], axis=AX.X, op=ALU.add)
                    rs = p2.tile([128, HH], f32, tag="rs")
                    nc.vector.reciprocal(rs[:], ssum[:])
                    rcol2 = p2.tile([128, HH], f32, tag="rcol2")
                    nc.vector.tensor_tensor(rcol2[:], rs[:], _rep(invDqs[s][:, nb : nb + 1], HH),
                                            op=ALU.mult)
                    nc.vector.tensor_tensor(alpha[:, nb, :, :], t1[:], _ins0(rs[:], M, -1),
                                            op=ALU.mult)
                    nc.vector.tensor_tensor(t1[:], t1[:], _ins0(rcol2[:], M, -1), op=ALU.mult)
                    for h in range(HH):
                        aT_ps = ps.tile([M, 128], f32, tag="ps")
                        nc.tensor.transpose(aT_ps[:], t1[:, h, :], identr[:])
                        nc.scalar.activation(alpha3T[:, h, nb * 128 : (nb + 1) * 128],
                                             aT_ps[:], AF.Copy, scale=invBs[s][:, 0:1])

                # ---- hypergraph conv: xl per head -> msg -> out_h ----
                msg = p1.tile([M, HH, C], f32r, tag="msg")
                for h in range(HH):
                    xlh = p2.tile([128, NB, C], f32r, tag="xlh")
                    for nb in range(NB):
                        xl_ps = ps.tile([128, C], f32, tag="ps")
                        for ct in range(CT):
                            nc.tensor.matmul(xl_ps[:],
                                             xhT[:, ct, nb * 128 : (nb + 1) * 128],
                                             wlin_t[:, ct, h * C : (h + 1) * C],
                                             start=(ct == 0), stop=(ct == CT - 1))
                        if h % 2 == 0:
                            nc.vector.tensor_copy(xlh[:, nb, :], xl_ps[:])
                        else:
                            nc.scalar.copy(xlh[:, nb, :], xl_ps[:])
                    msg_ps = ps.tile([M, C], f32, tag="ps")
                    for nb in range(NB):
                        nc.tensor.matmul(msg_ps[:], alpha[:, nb, h, :], xlh[:, nb, :],
                                         start=(nb == 0), stop=(nb == NB - 1))
                    if h % 2 == 0:
                        nc.scalar.copy(msg[:, h, :], msg_ps[:])
                    else:
                        nc.vector.tensor_copy(msg[:, h, :], msg_ps[:])

                outh_t = p1.tile([128, NB, C], f32r, tag="outh_t")
                for nb in range(NB):
                    oh_ps = ps.tile([128, C], f32, tag="ps")
                    for h in range(HH):
                        nc.tensor.matmul(oh_ps[:], alpha3T[:, h, nb * 128 : (nb + 1) * 128],
                                         msg[:, h, :], start=(h == 0), stop=False)
                    nc.tensor.matmul(oh_ps[:], ones_row[:], bhg_row[:], start=False, stop=True)
                    nc.scalar.activation(outh_t[:, nb, :], oh_ps[:], AF.Relu)

                # ---- RGCN: xw per relation -> aggregate + root + bias ----
                xw = p1.tile([128, NB, 2, C], f32r, tag="xw")
                for r in range(2):
                    for nb in range(NB):
                        xw_ps = ps.tile([128, C], f32, tag="ps")
                        for ct in range(CT):
                            nc.tensor.matmul(xw_ps[:],
                                             xrT[:, ct, nb * 128 : (nb + 1) * 128],
                                             wcat_t[:, ct, r * C : (r + 1) * C],
                                             start=(ct == 0), stop=(ct == CT - 1))
                        if (r + nb) % 2 == 0:
                            nc.vector.tensor_copy(xw[:, nb, r, :], xw_ps[:])
                        else:
                            nc.scalar.copy(xw[:, nb, r, :], xw_ps[:])

                outr_t = p1.tile([128, NB, C], f32r, tag="outr_t")
                for jb in range(NB):
                    a0_ps = ps.tile([128, C], f32, tag="ps")
                    for it in range(NB):
                        nc.tensor.matmul(a0_ps[:], Af[:, 0, it, jb * 128 : (jb + 1) * 128],
                                         xw[:, it, 0, :], start=(it == 0), stop=(it == NB - 1))
                    a1_ps = ps.tile([128, C], f32, tag="ps")
                    for it in range(NB):
                        nc.tensor.matmul(a1_ps[:], Af[:, 1, it, jb * 128 : (jb + 1) * 128],
                                         xw[:, it, 1, :], start=(it == 0), stop=(it == NB - 1))
                    rb_ps = ps.tile([128, C], f32, tag="ps")
                    for ct in range(CT):
                        nc.tensor.matmul(rb_ps[:],
                                         xrT[:, ct, jb * 128 : (jb + 1) * 128],
                                         wcat_t[:, ct, 2 * C : 3 * C],
                                         start=(ct == 0), stop=False)
                    nc.tensor.matmul(rb_ps[:], ones_row[:], brg_row[:], start=False, stop=True)
                    tb = p2.tile([128, C], f32, tag="tb")
                    nc.vector.tensor_scalar(tb[:], a0_ps[:], ivc[:, jb : jb + 1], None, op0=ALU.mult)
                    nc.vector.scalar_tensor_tensor(tb[:], a1_ps[:], ivc[:, NB + jb : NB + jb + 1],
                                                   tb[:], op0=ALU.mult, op1=ALU.add)
                    nc.vector.tensor_tensor(tb[:], rb_ps[:], tb[:], op=ALU.add)
                    nc.scalar.activation(outr_t[:, jb, :], tb[:], AF.Relu)

                # ---- gather ctx rows (row 0 of out_r / out_h) into bf16 ctxT columns ----
                ctx_psr = ps.tile([128, CT], f32, tag="ps")
                ctx_psh = ps.tile([128, CT], f32, tag="ps")
                for ct in range(CT):
                    nc.tensor.transpose(ctx_psr[:, ct : ct + 1],
                                        outr_t[0:1, 0, ct * 128 : (ct + 1) * 128].bitcast(f32),
                                        identr[0:1, 0:1])
                    nc.tensor.transpose(ctx_psh[:, ct : ct + 1],
                                        outh_t[0:1, 0, ct * 128 : (ct + 1) * 128].bitcast(f32),
                                        identr[0:1, 0:1])
                nc.vector.tensor_copy(ctxT[:, 0:CT, s], ctx_psr[:])
                nc.vector.tensor_copy(ctxT[:, CT : 2 * CT, s], ctx_psh[:])

                # ---- transpose outputs into next-layer feature-major state (layer 0) ----
                if l == 0:
                    xrT1 = xst.tile([128, CT, N], f32r, tag="xst")
                    xhT1 = xst.tile([128, CT, N], f32r, tag="xst")
                    for src, dst in ((outr_t, xrT1), (outh_t, xhT1)):
                        for nb in range(NB):
                            for ct in range(CT):
                                tp_ps = ps.tile([128, 128], f32, tag="ps")
                                nc.tensor.transpose(tp_ps[:],
                                                    src[:, nb, ct * 128 : (ct + 1) * 128].bitcast(f32),
                                                    identr[:])
                                if (nb + ct) % 2 == 0:
                                    nc.vector.tensor_copy(dst[:, ct, nb * 128 : (nb + 1) * 128], tp_ps[:])
                                else:
                                    nc.scalar.copy(dst[:, ct, nb * 128 : (nb + 1) * 128], tp_ps[:])
                    xrTs[s] = xrT1
                    xhTs[s] = xhT1
                else:
                    # final layer: store rows 1..N-1 (row 0 comes from the ie MLP below)
                    for tsrc, dram in ((outr_t, d_outr), (outh_t, d_outh)):
                        nc.sync.dma_start(dram[s, 1:128, :].bitcast(f32r), tsrc[1:128, 0, :])
                        nc.sync.dma_start(dram[s, 128:N, :].bitcast(f32r), tsrc[:, 1, :])

            # ---- info-exchange MLP over the 4 samples' ctx rows (bf16) ----
            y1 = p1.tile([BSL, C2], bf16, tag="y1")
            for ch in range(2):
                ie_ps = ps.tile([BSL, C], f32, tag="ps")
                for kt in range(KT2):
                    nc.tensor.matmul(ie_ps[:], ctxT[:, kt, :], iw1_t[:, kt, ch * C : (ch + 1) * C],
                                     start=(kt == 0), stop=False)
                nc.tensor.matmul(ie_ps[:], ones4b[:], ib1_row[:, ch * C : (ch + 1) * C],
                                 start=False, stop=True)
                nc.scalar.activation(y1[:, ch * C : (ch + 1) * C], ie_ps[:], AF.Relu)
            ctx2T = p1.tile([128, KT2, BSL], bf16, tag="ctx2T")
            for kt in range(KT2):
                t2_ps = ps.tile([128, BSL], bf16, tag="ps")
                nc.tensor.transpose(t2_ps[:], y1[:, kt * 128 : (kt + 1) * 128],
                                    identb[0:BSL, 0:BSL])
                nc.vector.tensor_copy(ctx2T[:, kt, :], t2_ps[:])
            y2 = p1.tile([BSL, C2], f32, tag="y2")
            for ch in range(2):
                ie2_ps = ps.tile([BSL, C], f32, tag="ps")
                for kt in range(KT2):
                    nc.tensor.matmul(ie2_ps[:], ctx2T[:, kt, :], iw2_t[:, kt, ch * C : (ch + 1) * C],
                                     start=(kt == 0), stop=False)
                nc.tensor.matmul(ie2_ps[:], ones4b[:], ib2_row[:, ch * C : (ch + 1) * C],
                                 start=False, stop=True)
                nc.vector.tensor_copy(y2[:, ch * C : (ch + 1) * C], ie2_ps[:])

            if l == 0:
                # scatter y2 rows back as column 0 of the next-layer feature-major states
                for kt in range(KT2):
                    y2T_ps = ps.tile([128, BSL], f32, tag="ps")
                    nc.tensor.transpose(y2T_ps[:], y2[:, kt * 128 : (kt + 1) * 128],
                                        identr[0:BSL, 0:BSL])
                    for s in range(BSL):
                        dst = xrTs[s] if kt < CT else xhTs[s]
                        nc.vector.tensor_copy(dst[:, kt % CT, 0:1], y2T_ps[:, s : s + 1])
            else:
                for s in range(BSL):
                    nc.sync.dma_start(d_outr[s, 0:1, :], y2[s : s + 1, 0:C])
                    nc.sync.dma_start(d_outh[s, 0:1, :], y2[s : s + 1, C:C2])

    nc.compile()
    return nc


_NC = None


def _get_nc():
    global _NC
    if _NC is None:
        _NC = build_module()
    return _NC


def make_in_maps(encoded_spans, SVO_emb, pooled_output, sent2word_adj, aug_adj,
                 punct_graph, w_rel, w_root, b_rgcn, w_lin, att_x, att_e, b_hgcn,
                 ie_w1, ie_b1, ie_w2, ie_b2):
    f = np.float32
    bf = ml_dtypes.bfloat16
    # host-folded attention vectors: u[c,h] = sum_k w_lin[c, h*C+k] * att[h,k]
    wl = np.ascontiguousarray(np.asarray(w_lin, f))                # [L, C, HH*C]
    wl4 = wl.reshape(L, C, HH, C)
    ux = np.einsum("lchk,lhk->lch", wl4, np.asarray(att_x, f))     # [L, C, HH]
    ue = np.einsum("lchk,lhk->lch", wl4, np.asarray(att_e, f))
    wr = np.asarray(w_rel, f)
    wcat = np.concatenate([wr[:, 0], wr[:, 1], np.asarray(w_root, f)], axis=2)
    e_attr = np.concatenate([np.asarray(pooled_output, f)[:, None, :],
                             np.asarray(SVO_emb, f)], axis=1)      # [BS, M, C]
    eaT = np.ascontiguousarray(e_attr.transpose(0, 2, 1))          # [BS, C, M]
    x0T = np.ascontiguousarray(np.asarray(encoded_spans, f).transpose(0, 2, 1))

    shared = {
        "wlin": wl,
        "ux": np.ascontiguousarray(ux),
        "ue": np.ascontiguousarray(ue),
        "wcat": np.ascontiguousarray(wcat),
        "iw1": np.asarray(ie_w1, f).astype(bf),
        "iw2": np.asarray(ie_w2, f).astype(bf),
        "brg": np.asarray(b_rgcn, f),
        "bhg": np.asarray(b_hgcn, f),
        "ib1": np.asarray(ie_b1, f).astype(bf),
        "ib2": np.asarray(ie_b2, f).astype(bf),
        "onesc": np.ones((1, 128), f),
        "eyer": np.eye(128, dtype=f),
        "eyeb": np.eye(128, dtype=f).astype(bf),
        "onesb": np.ones((1, 4), f).astype(bf),
    }
    s2w = np.ascontiguousarray(np.asarray(sent2word_adj, np.int32))
    aug = np.ascontiguousarray(np.asarray(aug_adj, np.int32))
    pun = np.ascontiguousarray(np.asarray(punct_graph, np.int32))

    in_maps = []
    for c in range(NCORES):
        sl = slice(c * BSL, (c + 1) * BSL)
        m = dict(shared)
        m["x0T"] = np.ascontiguousarray(x0T[sl])
        m["eaT"] = np.ascontiguousarray(eaT[sl])
        m["s2w"] = s2w[sl]
        m["aug"] = aug[sl]
        m["pun"] = pun[sl]
        in_maps.append(m)
    return in_maps


def run(in_maps, trace=False, **kw):
    nc = _get_nc()
    return run_bass_kernel_spmd(nc, in_maps, list(range(NCORES)), trace=trace, **kw)


def kernel(**inputs):
    in_maps = make_in_maps(**inputs)
    res = run(in_maps)
    x_r = np.concatenate([res.results[c]["outr"] for c in range(NCORES)], axis=0)
    x_h = np.concatenate([res.results[c]["outh"] for c in range(NCORES)], axis=0)
    return x_r.astype(np.float32), x_h.astype(np.float32)


# revision 11
# speedup vs baseline: 1.1203x; 1.0652x over previous
"""Trainium2 Bass kernel for nn_Message_gcn (2-layer RGCN + attention HypergraphConv + info-exchange MLP).

Sharding: pure data parallelism — batch 32 split as 4 samples on each of 8 NeuronCores,
per-layer weights replicated on every core.

Per-core algorithm (mathematically identical to the reference, restructured for the PE):
  - attention logits a_n / a_e computed via host-folded vectors u_x/u_e ([C,HH] = w_lin
    reshaped * att summed over the output channel), so the [M,C]@[C,4C] "el" matmul is
    never materialized.
  - softmax over incident hyperedges runs un-masked (max over all entries) and the mask
    is applied multiplicatively after exp() — identical by shift invariance since the
    global hyperedge keeps every row non-empty.
  - 1/deg, 1/|e|, 0.25/D(v) are folded into the adjacency columns / alpha operands so
    aggregations are plain matmuls with fp32 PSUM accumulation.
  - matmul operands use float32r (full-rate fp32 on the PE); the tiny info-exchange MLP
    (2 x [1024x1024] weights per layer, batched over the 4 samples) runs in bf16.
"""

import sys

sys.path.insert(0, "/opt/trn_rl_repo")

from contextlib import ExitStack

import numpy as np
import ml_dtypes

import concourse.bass as bass
import concourse.tile as tile
from concourse import bacc, mybir
from concourse.bass_utils import run_bass_kernel_spmd

BS, N, E, C, HH, L = 32, 256, 64, 512, 4, 2
M = E + 1
NCORES = 8
BSL = BS // NCORES          # samples per core
NB = N // 128               # node partition tiles
CT = C // 128               # channel partition tiles
C2 = 2 * C
KT2 = C2 // 128             # 2C partition tiles (ie)

f32 = mybir.dt.float32
f32r = mybir.dt.float32r
bf16 = mybir.dt.bfloat16
i32 = mybir.dt.int32
AF = mybir.ActivationFunctionType
ALU = mybir.AluOpType
AX = mybir.AxisListType


def _ins0(sl: bass.AP, count: int, pos: int) -> bass.AP:
    """Insert a 0-stride (broadcast) dim of `count` into an AP's free dims at
    position `pos` (0 = right after the partition dim, -1 = innermost)."""
    ap = [list(p) for p in sl.ap]
    if pos == -1:
        pos = len(ap) - 1
    ap.insert(1 + pos, [0, count])
    return bass.AP(tensor=sl.tensor, offset=sl.offset, ap=ap)


def _rep(sl: bass.AP, count: int) -> bass.AP:
    """[P, 1] AP -> [P, count] via 0-stride repeat of the free dim."""
    ap = [list(p) for p in sl.ap]
    assert ap[-1][1] == 1
    ap[-1] = [0, count]
    return bass.AP(tensor=sl.tensor, offset=sl.offset, ap=ap)


def _zeros_ap(sl: bass.AP, shape) -> bass.AP:
    """All-0-stride AP (scalar broadcast to `shape`, partition dim first)."""
    return bass.AP(tensor=sl.tensor, offset=sl.offset, ap=[[0, n] for n in shape])


def build_module():
    nc = bacc.Bacc("TRN2", target_bir_lowering=False, debug=False)

    # ---- DRAM I/O ----
    d_x0T = nc.dram_tensor("x0T", [BSL, C, N], f32r, kind="ExternalInput")
    d_eaT = nc.dram_tensor("eaT", [BSL, C, M], f32r, kind="ExternalInput")
    d_s2w = nc.dram_tensor("s2w", [BSL, N, E], i32, kind="ExternalInput")
    d_aug = nc.dram_tensor("aug", [BSL, N, N], i32, kind="ExternalInput")
    d_pun = nc.dram_tensor("pun", [BSL, N, N], i32, kind="ExternalInput")
    d_wlin = nc.dram_tensor("wlin", [L, C, HH * C], f32r, kind="ExternalInput")
    d_ux = nc.dram_tensor("ux", [L, C, HH], f32r, kind="ExternalInput")
    d_ue = nc.dram_tensor("ue", [L, C, HH], f32r, kind="ExternalInput")
    d_wcat = nc.dram_tensor("wcat", [L, C, 3 * C], f32r, kind="ExternalInput")
    d_iw1 = nc.dram_tensor("iw1", [L, C2, C2], bf16, kind="ExternalInput")
    d_iw2 = nc.dram_tensor("iw2", [L, C2, C2], bf16, kind="ExternalInput")
    d_brg = nc.dram_tensor("brg", [L, C], f32r, kind="ExternalInput")
    d_bhg = nc.dram_tensor("bhg", [L, C], f32r, kind="ExternalInput")
    d_ib1 = nc.dram_tensor("ib1", [L, C2], bf16, kind="ExternalInput")
    d_ib2 = nc.dram_tensor("ib2", [L, C2], bf16, kind="ExternalInput")
    # host constants (memset cannot write f32r)
    d_ones = nc.dram_tensor("onesc", [1, 128], f32r, kind="ExternalInput")
    d_eyer = nc.dram_tensor("eyer", [128, 128], f32r, kind="ExternalInput")
    d_eyeb = nc.dram_tensor("eyeb", [128, 128], bf16, kind="ExternalInput")
    d_onesb = nc.dram_tensor("onesb", [1, 4], bf16, kind="ExternalInput")
    d_outr = nc.dram_tensor("outr", [BSL, N, C], f32, kind="ExternalOutput")
    d_outh = nc.dram_tensor("outh", [BSL, N, C], f32, kind="ExternalOutput")

    with ExitStack() as ctx:
        tc = ctx.enter_context(tile.TileContext(nc))
        const = ctx.enter_context(tc.tile_pool(name="const", bufs=1))
        wts = ctx.enter_context(tc.tile_pool(name="wts", bufs=1))
        xst = ctx.enter_context(tc.tile_pool(name="xst", bufs=9))
        graph = ctx.enter_context(tc.tile_pool(name="graph", bufs=BSL))
        p1 = ctx.enter_context(tc.tile_pool(name="p1", bufs=1))
        p2 = ctx.enter_context(tc.tile_pool(name="p2", bufs=2))
        ps = ctx.enter_context(tc.tile_pool(name="ps", bufs=8, space="PSUM"))

        # ---- constants ----
        ones_row = const.tile([1, 128], f32r)
        nc.sync.dma_start(ones_row[:], d_ones[:])
        ones_col = const.tile([128, 1], f32r)
        nc.sync.dma_start(ones_col[:], d_ones[0:1, :].rearrange("o p -> p o"))
        identr = const.tile([128, 128], f32)
        nc.sync.dma_start(identr[:], d_eyer[:].bitcast(f32))
        identb = const.tile([128, 128], bf16)
        nc.sync.dma_start(identb[:], d_eyeb[:])
        ones4b = const.tile([1, 4], bf16)
        nc.sync.dma_start(ones4b[:], d_onesb[:])

        # ---- per-sample, layer-invariant graph state ----
        Hincs, invDqs, invBs, x0Ts = [], [], [], []
        for s in range(BSL):
            x0T_t = xst.tile([128, CT, N], f32r, tag="xst")
            nc.scalar.dma_start(x0T_t[:], d_x0T[s].rearrange("(ct p) n -> p ct n", p=128))
            x0Ts.append(x0T_t)

            Hinc_t = graph.tile([128, NB, M], f32r)
            # col 0 (global hyperedge) = 1.0, cols 1..M-1 = sent2word incidence
            nc.sync.dma_start(Hinc_t[:, :, 0:1], _zeros_ap(d_ones[0:1, 0:1], [128, NB, 1]))
            nc.gpsimd.dma_start(Hinc_t[:, :, 1:M], d_s2w[s].rearrange("(t p) e -> p t e", p=128))
            Hincs.append(Hinc_t)

            # node degree -> 0.25/D(v) (head-mean folded in)
            Dn = p2.tile([128, NB], f32, tag="Dn")
            nc.vector.tensor_reduce(Dn[:], Hinc_t[:], axis=AX.X, op=ALU.add)
            eqD = p2.tile([128, NB], f32, tag="eqD")
            nc.vector.tensor_scalar(eqD[:], Dn[:], 0.0, None, op0=ALU.is_equal)
            invDq_t = graph.tile([128, NB], f32)
            nc.vector.tensor_add(invDq_t[:], Dn[:], eqD[:])
            nc.vector.reciprocal(invDq_t[:], invDq_t[:])
            nc.vector.tensor_sub(invDq_t[:], invDq_t[:], eqD[:])
            nc.vector.tensor_scalar(invDq_t[:], invDq_t[:], 0.25, None, op0=ALU.mult)
            invDqs.append(invDq_t)

            # hyperedge size -> 1/|e|
            Be_ps = ps.tile([M, 1], f32, tag="ps")
            for it in range(NB):
                nc.tensor.matmul(Be_ps[:], Hinc_t[:, it, :].bitcast(f32), ones_col[:].bitcast(f32),
                                 start=(it == 0), stop=(it == NB - 1))
            Be = p2.tile([M, 1], f32, tag="Be")
            nc.vector.tensor_copy(Be[:], Be_ps[:])
            eqB = p2.tile([M, 1], f32, tag="eqB")
            nc.vector.tensor_scalar(eqB[:], Be[:], 0.0, None, op0=ALU.is_equal)
            invB_t = graph.tile([M, 1], f32)
            nc.vector.tensor_add(invB_t[:], Be[:], eqB[:])
            nc.vector.reciprocal(invB_t[:], invB_t[:])
            nc.vector.tensor_sub(invB_t[:], invB_t[:], eqB[:])
            invBs.append(invB_t)

        xrTs = list(x0Ts)
        xhTs = list(x0Ts)

        for l in range(L):
            # ---- layer weights ----
            wlin_t = wts.tile([128, CT, HH * C], f32r, tag="wlin")
            nc.scalar.dma_start(wlin_t[:], d_wlin[l].rearrange("(ct p) k -> p ct k", p=128))
            ux_t = wts.tile([128, CT, HH], f32r, tag="ux")
            nc.sync.dma_start(ux_t[:], d_ux[l].rearrange("(ct p) h -> p ct h", p=128))
            ue_t = wts.tile([128, CT, HH], f32r, tag="ue")
            nc.sync.dma_start(ue_t[:], d_ue[l].rearrange("(ct p) h -> p ct h", p=128))
            wcat_t = wts.tile([128, CT, 3 * C], f32r, tag="wcat")
            nc.sync.dma_start(wcat_t[:], d_wcat[l].rearrange("(ct p) k -> p ct k", p=128))
            iw1_t = wts.tile([128, KT2, C2], bf16, tag="iew")
            nc.scalar.dma_start(iw1_t[:], d_iw1[l].rearrange("(kt p) k -> p kt k", p=128))
            iw2_t = wts.tile([128, KT2, C2], bf16, tag="iew")
            nc.scalar.dma_start(iw2_t[:], d_iw2[l].rearrange("(kt p) k -> p kt k", p=128))
            brg_row = wts.tile([1, C], f32r, tag="brg")
            nc.sync.dma_start(brg_row[:], d_brg[l : l + 1, :])
            bhg_row = wts.tile([1, C], f32r, tag="bhg")
            nc.sync.dma_start(bhg_row[:], d_bhg[l : l + 1, :])
            ib1_row = wts.tile([1, C2], bf16, tag="ib1")
            nc.sync.dma_start(ib1_row[:], d_ib1[l : l + 1, :])
            ib2_row = wts.tile([1, C2], bf16, tag="ib2")
            nc.sync.dma_start(ib2_row[:], d_ib2[l : l + 1, :])

            ctxT = p1.tile([128, 2 * CT, BSL], bf16, tag="ctxT")
            for s in range(BSL):
                xrT = xrTs[s]
                xhT = xhTs[s]

                # ---- typed adjacency A' with 1/deg folded into its columns ----
                Af = p2.tile([128, 2, NB, N], f32r, tag="Af")
                nc.gpsimd.dma_start(Af[:, 1, :, :], d_aug[s].rearrange("(t p) j -> p t j", p=128))
                nc.gpsimd.dma_start(Af[:, 0, :, :], d_pun[s].rearrange("(t p) j -> p t j", p=128))
                onem = p1.tile([128, NB, N], f32, tag="gtmp")
                nc.vector.tensor_scalar(onem[:], Af[:, 1, :, :], -1.0, 1.0, op0=ALU.mult, op1=ALU.add)
                nc.vector.tensor_mul(Af[:, 0, :, :], Af[:, 0, :, :], onem[:])

                # in-degree rows -> transpose to per-target columns; guarded 1/deg
                deg_ps = ps.tile([1, 2, N], f32, tag="ps")
                for r in range(2):
                    for it in range(NB):
                        nc.tensor.matmul(deg_ps[:, r, :], ones_col[:], Af[:, r, it, :],
                                         start=(it == 0), stop=(it == NB - 1))
                degrow = p1.tile([1, 2, N], f32, tag="degrow")
                nc.scalar.copy(degrow[:], deg_ps[:])
                degc_ps = ps.tile([128, 2 * NB], f32, tag="ps")
                for r in range(2):
                    for jb in range(NB):
                        nc.tensor.transpose(degc_ps[:, r * NB + jb : r * NB + jb + 1],
                                            degrow[0:1, r, jb * 128 : (jb + 1) * 128],
                                            identr[0:1, 0:1])
                eqc = p2.tile([128, 2 * NB], f32, tag="eqc")
                nc.vector.tensor_scalar(eqc[:], degc_ps[:], 0.0, None, op0=ALU.is_equal)
                ivc = p2.tile([128, 2 * NB], f32, tag="ivc")
                nc.vector.tensor_add(ivc[:], degc_ps[:], eqc[:])
                nc.vector.reciprocal(ivc[:], ivc[:])
                nc.vector.tensor_sub(ivc[:], ivc[:], eqc[:])

                # ---- attention logits ----
                an_ps = ps.tile([128, NB * HH], f32, tag="ps")
                for nb in range(NB):
                    for ct in range(CT):
                        nc.tensor.matmul(an_ps[:, nb * HH : (nb + 1) * HH],
                                         xhT[:, ct, nb * 128 : (nb + 1) * 128].bitcast(f32),
                                         ux_t[:, ct, :].bitcast(f32),
                                         start=(ct == 0), stop=(ct == CT - 1))
                an_sb = p2.tile([128, NB * HH], f32, tag="ansb")
                nc.vector.tensor_copy(an_sb[:], an_ps[:])

                eaT_t = p2.tile([128, CT, M], f32r, tag="eaT")
                nc.sync.dma_start(eaT_t[:], d_eaT[s].rearrange("(ct p) m -> p ct m", p=128))
                ae_ps = ps.tile([HH, M], f32, tag="ps")
                for ct in range(CT):
                    nc.tensor.matmul(ae_ps[:], ue_t[:, ct, :].bitcast(f32),
                                     eaT_t[:, ct, :].bitcast(f32),
                                     start=(ct == 0), stop=(ct == CT - 1))
                ae4_sb = p2.tile([HH, M], f32r, tag="ae4sb")
                nc.vector.tensor_copy(ae4_sb[:], ae_ps[:])
                # gather the 4 head rows onto partition 0 (matmul base-partition rule)
                ae_row = p2.tile([1, HH, M], f32r, tag="aerow")
                for h in range(HH):
                    nc.scalar.dma_start(ae_row[:, h, :], ae4_sb[h : h + 1, :])
                # a_e broadcast along nodes in one K=1 fp32r matmul (shared by both node blocks)
                ab_ps = ps.tile([128, HH, M], f32, tag="ps")
                nc.tensor.matmul(ab_ps[:], ones_row[:], ae_row[0:1, :, :], start=True, stop=True)

                # ---- alpha (leaky relu + masked softmax), invD/invB folded variants ----
                alpha = p2.tile([128, NB, HH, M], f32r, tag="alpha")
                alpha3T = p2.tile([M, HH, N], f32r, tag="alpha3T")
                for nb in range(NB):
                    t1 = p2.tile([128, HH, M], f32, tag="t1")
                    nc.vector.tensor_tensor(t1[:], ab_ps[:],
                                            _ins0(an_sb[:, nb * HH : (nb + 1) * HH], M, -1),
                                            op=ALU.add)
                    nc.vector.scalar_tensor_tensor(t1[:], t1[:], 0.2, t1[:],
                                                   op0=ALU.mult, op1=ALU.max)
                    nmax = p2.tile([128, HH], f32, tag="nmax")
                    nc.vector.tensor_reduce(nmax[:], t1[:], axis=AX.X, op=ALU.max, negate=True)
                    nc.vector.tensor_tensor(t1[:], t1[:], _ins0(nmax[:], M, -1), op=ALU.add)
                    nc.scalar.activation(t1[:], t1[:], AF.Exp)
                    nc.vector.tensor_tensor(t1[:], t1[:], _ins0(Hincs[s][:, nb, :], HH, 0),
                                            op=ALU.mult)
                    ssum = p2.tile([128, HH], f32, tag="ssum")
                    nc.vector.tensor_reduce(ssum[:], t1[:], axis=AX.X, op=ALU.add)
                    rs = p2.tile([128, HH], f32, tag="rs")
                    nc.vector.reciprocal(rs[:], ssum[:])
                    rcol2 = p2.tile([128, HH], f32, tag="rcol2")
                    nc.vector.tensor_tensor(rcol2[:], rs[:], _rep(invDqs[s][:, nb : nb + 1], HH),
                                            op=ALU.mult)
                    nc.vector.tensor_tensor(alpha[:, nb, :, :], t1[:], _ins0(rs[:], M, -1),
                                            op=ALU.mult)
                    nc.vector.tensor_tensor(t1[:], t1[:], _ins0(rcol2[:], M, -1), op=ALU.mult)
                    for h in range(HH):
                        aT_ps = ps.tile([M, 128], f32, tag="ps")
                        nc.tensor.transpose(aT_ps[:], t1[:, h, :], identr[:])
                        nc.scalar.activation(alpha3T[:, h, nb * 128 : (nb + 1) * 128],
                                             aT_ps[:], AF.Copy, scale=invBs[s][:, 0:1])

                # ---- hypergraph conv: xl per head -> msg -> out_h ----
                msg = p2.tile([M, HH, C], f32r, tag="msg")
                for h in range(HH):
                    xlh = p2.tile([128, NB, C], f32r, tag="xlh")
                    for nb in range(NB):
                        xl_ps = ps.tile([128, C], f32, tag="ps")
                        for ct in range(CT):
                            nc.tensor.matmul(xl_ps[:],
                                             xhT[:, ct, nb * 128 : (nb + 1) * 128],
                                             wlin_t[:, ct, h * C : (h + 1) * C],
                                             start=(ct == 0), stop=(ct == CT - 1))
                        if h % 2 == 0:
                            nc.vector.tensor_copy(xlh[:, nb, :], xl_ps[:])
                        else:
                            nc.scalar.copy(xlh[:, nb, :], xl_ps[:])
                    msg_ps = ps.tile([M, C], f32, tag="ps")
                    for nb in range(NB):
                        nc.tensor.matmul(msg_ps[:], alpha[:, nb, h, :], xlh[:, nb, :],
                                         start=(nb == 0), stop=(nb == NB - 1))
                    if h % 2 == 0:
                        nc.scalar.copy(msg[:, h, :], msg_ps[:])
                    else:
                        nc.vector.tensor_copy(msg[:, h, :], msg_ps[:])

                outh_t = p1.tile([128, NB, C], f32r, tag="outh_t")
                for nb in range(NB):
                    oh_ps = ps.tile([128, C], f32, tag="ps")
                    for h in range(HH):
                        nc.tensor.matmul(oh_ps[:], alpha3T[:, h, nb * 128 : (nb + 1) * 128],
                                         msg[:, h, :], start=(h == 0), stop=False)
                    nc.tensor.matmul(oh_ps[:], ones_row[:], bhg_row[:], start=False, stop=True)
                    nc.scalar.activation(outh_t[:, nb, :], oh_ps[:], AF.Relu)

                # ---- RGCN: xw per relation -> aggregate + root + bias ----
                xw = p2.tile([128, NB, 2, C], f32r, tag="xw")
                for r in range(2):
                    for nb in range(NB):
                        xw_ps = ps.tile([128, C], f32, tag="ps")
                        for ct in range(CT):
                            nc.tensor.matmul(xw_ps[:],
                                             xrT[:, ct, nb * 128 : (nb + 1) * 128],
                                             wcat_t[:, ct, r * C : (r + 1) * C],
                                             start=(ct == 0), stop=(ct == CT - 1))
                        if (r + nb) % 2 == 0:
                            nc.vector.tensor_copy(xw[:, nb, r, :], xw_ps[:])
                        else:
                            nc.scalar.copy(xw[:, nb, r, :], xw_ps[:])

                outr_t = p1.tile([128, NB, C], f32r, tag="outr_t")
                for jb in range(NB):
                    a0_ps = ps.tile([128, C], f32, tag="ps")
                    for it in range(NB):
                        nc.tensor.matmul(a0_ps[:], Af[:, 0, it, jb * 128 : (jb + 1) * 128],
                                         xw[:, it, 0, :], start=(it == 0), stop=(it == NB - 1))
                    a1_ps = ps.tile([128, C], f32, tag="ps")
                    for it in range(NB):
                        nc.tensor.matmul(a1_ps[:], Af[:, 1, it, jb * 128 : (jb + 1) * 128],
                                         xw[:, it, 1, :], start=(it == 0), stop=(it == NB - 1))
                    rb_ps = ps.tile([128, C], f32, tag="ps")
                    for ct in range(CT):
                        nc.tensor.matmul(rb_ps[:],
                                         xrT[:, ct, jb * 128 : (jb + 1) * 128],
                                         wcat_t[:, ct, 2 * C : 3 * C],
                                         start=(ct == 0), stop=False)
                    nc.tensor.matmul(rb_ps[:], ones_row[:], brg_row[:], start=False, stop=True)
                    tb = p1.tile([128, C], f32, tag="tb")
                    nc.vector.tensor_scalar(tb[:], a0_ps[:], ivc[:, jb : jb + 1], None, op0=ALU.mult)
                    nc.vector.scalar_tensor_tensor(tb[:], a1_ps[:], ivc[:, NB + jb : NB + jb + 1],
                                                   tb[:], op0=ALU.mult, op1=ALU.add)
                    nc.vector.tensor_tensor(tb[:], rb_ps[:], tb[:], op=ALU.add)
                    nc.scalar.activation(outr_t[:, jb, :], tb[:], AF.Relu)

                # ---- gather ctx rows (row 0 of out_r / out_h) into bf16 ctxT columns ----
                ctx_psr = ps.tile([128, CT], f32, tag="ps")
                ctx_psh = ps.tile([128, CT], f32, tag="ps")
                for ct in range(CT):
                    nc.tensor.transpose(ctx_psr[:, ct : ct + 1],
                                        outr_t[0:1, 0, ct * 128 : (ct + 1) * 128].bitcast(f32),
                                        identr[0:1, 0:1])
                    nc.tensor.transpose(ctx_psh[:, ct : ct + 1],
                                        outh_t[0:1, 0, ct * 128 : (ct + 1) * 128].bitcast(f32),
                                        identr[0:1, 0:1])
                nc.vector.tensor_copy(ctxT[:, 0:CT, s], ctx_psr[:])
                nc.vector.tensor_copy(ctxT[:, CT : 2 * CT, s], ctx_psh[:])

                # ---- transpose outputs into next-layer feature-major state (layer 0) ----
                if l == 0:
                    xrT1 = xst.tile([128, CT, N], f32r, tag="xst")
                    xhT1 = xst.tile([128, CT, N], f32r, tag="xst")
                    for src, dst in ((outr_t, xrT1), (outh_t, xhT1)):
                        for nb in range(NB):
                            for ct in range(CT):
                                tp_ps = ps.tile([128, 128], f32, tag="ps")
                                nc.tensor.transpose(tp_ps[:],
                                                    src[:, nb, ct * 128 : (ct + 1) * 128].bitcast(f32),
                                                    identr[:])
                                if (nb + ct) % 2 == 0:
                                    nc.vector.tensor_copy(dst[:, ct, nb * 128 : (nb + 1) * 128], tp_ps[:])
                                else:
                                    nc.scalar.copy(dst[:, ct, nb * 128 : (nb + 1) * 128], tp_ps[:])
                    xrTs[s] = xrT1
                    xhTs[s] = xhT1
                else:
                    # final layer: store rows 1..N-1 (row 0 comes from the ie MLP below)
                    for tsrc, dram in ((outr_t, d_outr), (outh_t, d_outh)):
                        nc.sync.dma_start(dram[s, 1:128, :].bitcast(f32r), tsrc[1:128, 0, :])
                        nc.sync.dma_start(dram[s, 128:N, :].bitcast(f32r), tsrc[:, 1, :])

            # ---- info-exchange MLP over the 4 samples' ctx rows (bf16) ----
            y1 = p1.tile([BSL, C2], bf16, tag="y1")
            for ch in range(2):
                ie_ps = ps.tile([BSL, C], f32, tag="ps")
                for kt in range(KT2):
                    nc.tensor.matmul(ie_ps[:], ctxT[:, kt, :], iw1_t[:, kt, ch * C : (ch + 1) * C],
                                     start=(kt == 0), stop=False)
                nc.tensor.matmul(ie_ps[:], ones4b[:], ib1_row[:, ch * C : (ch + 1) * C],
                                 start=False, stop=True)
                nc.scalar.activation(y1[:, ch * C : (ch + 1) * C], ie_ps[:], AF.Relu)
            ctx2T = p1.tile([128, KT2, BSL], bf16, tag="ctx2T")
            for kt in range(KT2):
                t2_ps = ps.tile([128, BSL], bf16, tag="ps")
                nc.tensor.transpose(t2_ps[:], y1[:, kt * 128 : (kt + 1) * 128],
                                    identb[0:BSL, 0:BSL])
                nc.vector.tensor_copy(ctx2T[:, kt, :], t2_ps[:])
            y2 = p1.tile([BSL, C2], f32, tag="y2")
            for ch in range(2):
                ie2_ps = ps.tile([BSL, C], f32, tag="ps")
                for kt in range(KT2):
                    nc.tensor.matmul(ie2_ps[:], ctx2T[:, kt, :], iw2_t[:, kt, ch * C : (ch + 1) * C],
                                     start=(kt == 0), stop=False)
                nc.tensor.matmul(ie2_ps[:], ones4b[:], ib2_row[:, ch * C : (ch + 1) * C],
                                 start=False, stop=True)
                nc.vector.tensor_copy(y2[:, ch * C : (ch + 1) * C], ie2_ps[:])

            if l == 0:
                # scatter y2 rows back as column 0 of the next-layer feature-major states
                for kt in range(KT2):
                    y2T_ps = ps.tile([128, BSL], f32, tag="ps")
                    nc.tensor.transpose(y2T_ps[:], y2[:, kt * 128 : (kt + 1) * 128],
                                        identr[0:BSL, 0:BSL])
                    for s in range(BSL):
                        dst = xrTs[s] if kt < CT else xhTs[s]
                        nc.vector.tensor_copy(dst[:, kt % CT, 0:1], y2T_ps[:, s : s + 1])
            else:
                for s in range(BSL):
                    nc.sync.dma_start(d_outr[s, 0:1, :], y2[s : s + 1, 0:C])
                    nc.sync.dma_start(d_outh[s, 0:1, :], y2[s : s + 1, C:C2])

    nc.compile()
    return nc


_NC = None


def _get_nc():
    global _NC
    if _NC is None:
        _NC = build_module()
    return _NC


def make_in_maps(encoded_spans, SVO_emb, pooled_output, sent2word_adj, aug_adj,
                 punct_graph, w_rel, w_root, b_rgcn, w_lin, att_x, att_e, b_hgcn,
                 ie_w1, ie_b1, ie_w2, ie_b2):
    f = np.float32
    bf = ml_dtypes.bfloat16
    # host-folded attention vectors: u[c,h] = sum_k w_lin[c, h*C+k] * att[h,k]
    wl = np.ascontiguousarray(np.asarray(w_lin, f))                # [L, C, HH*C]
    wl4 = wl.reshape(L, C, HH, C)
    ux = np.einsum("lchk,lhk->lch", wl4, np.asarray(att_x, f))     # [L, C, HH]
    ue = np.einsum("lchk,lhk->lch", wl4, np.asarray(att_e, f))
    wr = np.asarray(w_rel, f)
    wcat = np.concatenate([wr[:, 0], wr[:, 1], np.asarray(w_root, f)], axis=2)
    e_attr = np.concatenate([np.asarray(pooled_output, f)[:, None, :],
                             np.asarray(SVO_emb, f)], axis=1)      # [BS, M, C]
    eaT = np.ascontiguousarray(e_attr.transpose(0, 2, 1))          # [BS, C, M]
    x0T = np.ascontiguousarray(np.asarray(encoded_spans, f).transpose(0, 2, 1))

    shared = {
        "wlin": wl,
        "ux": np.ascontiguousarray(ux),
        "ue": np.ascontiguousarray(ue),
        "wcat": np.ascontiguousarray(wcat),
        "iw1": np.asarray(ie_w1, f).astype(bf),
        "iw2": np.asarray(ie_w2, f).astype(bf),
        "brg": np.asarray(b_rgcn, f),
        "bhg": np.asarray(b_hgcn, f),
        "ib1": np.asarray(ie_b1, f).astype(bf),
        "ib2": np.asarray(ie_b2, f).astype(bf),
        "onesc": np.ones((1, 128), f),
        "eyer": np.eye(128, dtype=f),
        "eyeb": np.eye(128, dtype=f).astype(bf),
        "onesb": np.ones((1, 4), f).astype(bf),
    }
    s2w = np.ascontiguousarray(np.asarray(sent2word_adj, np.int32))
    aug = np.ascontiguousarray(np.asarray(aug_adj, np.int32))
    pun = np.ascontiguousarray(np.asarray(punct_graph, np.int32))

    in_maps = []
    for c in range(NCORES):
        sl = slice(c * BSL, (c + 1) * BSL)
        m = dict(shared)
        m["x0T"] = np.ascontiguousarray(x0T[sl])
        m["eaT"] = np.ascontiguousarray(eaT[sl])
        m["s2w"] = s2w[sl]
        m["aug"] = aug[sl]
        m["pun"] = pun[sl]
        in_maps.append(m)
    return in_maps


def run(in_maps, trace=False, **kw):
    nc = _get_nc()
    return run_bass_kernel_spmd(nc, in_maps, list(range(NCORES)), trace=trace, **kw)


def kernel(**inputs):
    in_maps = make_in_maps(**inputs)
    res = run(in_maps)
    x_r = np.concatenate([res.results[c]["outr"] for c in range(NCORES)], axis=0)
    x_h = np.concatenate([res.results[c]["outh"] for c in range(NCORES)], axis=0)
    return x_r.astype(np.float32), x_h.astype(np.float32)
